# revision 1
# baseline (speedup 1.0000x reference)
"""HMM window log-likelihood on 8 NeuronCores (data-parallel over batch).

Math: reference computes, per batch column b,
    y[b] = exp(logsumexp_i x_T[b,i]),  x via log-space forward recursion.
Equivalently in linear space with row-normalized transition matrices
W_t = exp(w[t-1]) / rowsum, emission table L = softmax(distros, axis=1):
    y[b] = 1^T diag(em_T) W_T ... diag(em_1) W_1 em_0
We evaluate it as a BACKWARD recursion (avoids transposing W on device):
    beta_L = 1;  beta_{t-1} = W_t^T (em_t . beta_t)
    y[b] = sum_i em_0[i,b] beta_0[i,b]
with per-step rescale factors g_t (host-computed from column 0, f64) folded
into recipSg[:,t] = g_t / rowsum_t to keep everything in bf16/f32 range.
em_t[i,b] = L[i, bin(b,t)] is computed on the PE as dLT^T @ G_t where
dL[i,k] = L[i,k]-L[i,k-1] and G_t[k,b] = [bin(b,t) >= k] (0/1 indicators).
Device returns colsum[b] = y[b] * prod(g); host: lnY = log(colsum)+C, y=exp.
The true lnY is ~ -584.6 for these inputs, so y underflows f32 to 0.0 —
exactly matching the reference (which also underflows in f32).
"""
import sys, os
for p in ("/opt/trn_rl_repo",):
    if p not in sys.path:
        sys.path.insert(0, p)
import numpy as np
import ml_dtypes

from concourse import bass, bacc, mybir
from concourse.tile import TileContext
from concourse.bass_utils import run_bass_kernel_spmd

W, L, B, NB = 128, 256, 4096, 10
NCORES = 8
BC = B // NCORES          # 512 batch cols per core
BH = BC // 2              # two half-chains of 256
TBLK = 16                 # G streaming block (t's per DMA)

LAST_LNY = None           # debug: device-derived lnY per batch col
LAST_RESULTS = None       # debug: raw BassKernelResults

_CACHED = None            # (nc,) build cache


def _build_nc():
    nc = bacc.Bacc("TRN2", target_bir_lowering=False, debug=False,
                   num_devices=NCORES)
    bf16, f32 = mybir.dt.bfloat16, mybir.dt.float32

    wt = nc.dram_tensor("wt", [W, L - 1, W], bf16, kind="ExternalInput")
    dlt = nc.dram_tensor("dlt", [NB, W], bf16, kind="ExternalInput")
    rsg = nc.dram_tensor("rsg", [W, L], f32, kind="ExternalInput")
    g10 = nc.dram_tensor("g10", [NB, L, BC], bf16, kind="ExternalInput")
    ones = nc.dram_tensor("ones", [W, 1], bf16, kind="ExternalInput")
    colsum = nc.dram_tensor("colsum", [1, BC], f32, kind="ExternalOutput")

    Copy = mybir.ActivationFunctionType.Copy

    with TileContext(nc) as tc:
        with tc.sbuf_pool(name="sb", bufs=2) as sb, \
                tc.psum_pool(name="ps", bufs=2) as ps:
            dlt_sb = sb.tile([NB, W], bf16, bufs=1)
            nc.sync.dma_start(dlt_sb, dlt.ap())
            rsg_sb = sb.tile([W, L], f32, bufs=1)
            nc.sync.dma_start(rsg_sb, rsg.ap())
            ones_sb = sb.tile([W, 1], bf16, bufs=1)
            nc.sync.dma_start(ones_sb, ones.ap())

            # all 255 transition matrices resident; chunked DMAs in backward
            # order so the scan can start as soon as the tail chunk lands
            wt_sb = sb.tile([W, L - 1, W], bf16, bufs=1)
            for cc in range((L - 1 + 7) // 8 - 1, -1, -1):
                t0 = cc * 8
                cnt = min(8, L - 1 - t0)
                nc.sync.dma_start(wt_sb[:, t0:t0 + cnt, :],
                                  wt.ap()[:, t0:t0 + cnt, :])

            cs_ps = None
            beta_ps = [None, None]
            for blk in range(L // TBLK - 1, -1, -1):
                g_sb = sb.tile([NB, TBLK, BC], bf16, tag="g", bufs=3)
                nc.sync.dma_start(
                    g_sb, g10.ap()[:, blk * TBLK:(blk + 1) * TBLK, :])
                for ti in range(TBLK - 1, -1, -1):
                    t = blk * TBLK + ti
                    for h in (0, 1):
                        em_ps = ps.tile([W, BH], f32, tag=f"em{h}", bufs=2)
                        nc.tensor.matmul(
                            em_ps, dlt_sb,
                            g_sb[:, ti, h * BH:(h + 1) * BH],
                            start=True, stop=True)
                        em_sb = sb.tile([W, BH], bf16, tag=f"emsb{h}", bufs=3)
                        nc.scalar.activation(em_sb, em_ps, Copy,
                                             scale=rsg_sb[:, t:t + 1])
                        if t == L - 1:
                            c_sb = em_sb
                        else:
                            c_sb = sb.tile([W, BH], bf16, tag=f"c{h}", bufs=3)
                            nc.vector.tensor_mul(c_sb, beta_ps[h], em_sb)
                        if t > 0:
                            b_ps = ps.tile([W, BH], f32, tag=f"b{h}", bufs=2)
                            nc.tensor.matmul(b_ps, wt_sb[:, t - 1, :], c_sb,
                                             start=True, stop=True)
                            beta_ps[h] = b_ps
                        else:
                            if cs_ps is None:
                                cs_ps = ps.tile([1, BC], f32, tag="em0",
                                                bufs=2)
                            nc.tensor.matmul(cs_ps[:, h * BH:(h + 1) * BH],
                                             ones_sb, c_sb,
                                             start=True, stop=True)

            cs_sb = sb.tile([1, BC], f32, bufs=1)
            nc.vector.tensor_copy(cs_sb, cs_ps)
            nc.sync.dma_start(colsum.ap(), cs_sb)
    nc.compile()
    return nc


def _host_prep(data, input_distros, dense_layer_weights):
    f64 = np.float64
    we = np.exp(dense_layer_weights.astype(f64))           # (255,W,W)
    rowsum = we.sum(axis=2)                                # (255,W)
    recip = 1.0 / rowsum
    d = input_distros.astype(f64)
    d = d - d.max(axis=1, keepdims=True)
    e = np.exp(d)
    Ll = e / e.sum(axis=1, keepdims=True)                  # (W,NB) softmax rows
    # bins exactly as reference: floor(v / 0.1) in f32
    bins = np.minimum(NB - 1, np.floor(
        data / np.float32(0.1)).astype(np.int32))          # (B,L)

    # column-0 f64 backward pass -> per-step rescale g_t, offset C
    beta = np.ones(W, dtype=f64)
    Cacc = 0.0
    g = np.ones(L, dtype=f64)
    for t in range(L - 1, 0, -1):
        c = Ll[np.arange(W), bins[0, t]] * beta * recip[t - 1]
        tmp = we[t - 1].T @ c
        f = tmp.max()
        g[t] = 1.0 / f
        Cacc += np.log(f)
        beta = tmp * g[t]

    rsg = np.ones((W, L), dtype=np.float32)
    rsg[:, 1:] = (recip.T * g[None, 1:]).astype(np.float32)

    dL = Ll.copy()
    dL[:, 1:] -= Ll[:, :-1]
    dlt = np.ascontiguousarray(dL.T).astype(ml_dtypes.bfloat16)  # (NB,W)

    wt = np.ascontiguousarray(
        we.transpose(1, 0, 2)).astype(ml_dtypes.bfloat16)  # (W,255,W)

    # G[k,t,b] = [bins[b,t] >= k]   (G[0] == 1)
    g10 = (bins.T[None, :, :] >= np.arange(NB)[:, None, None]
           ).astype(ml_dtypes.bfloat16)                    # (NB,L,B)
    ones_v = np.ones((W, 1), dtype=ml_dtypes.bfloat16)
    return wt, dlt, rsg, g10, ones_v, Cacc


def kernel(data, input_distros, dense_layer_weights):
    global LAST_LNY, LAST_RESULTS, _CACHED
    wt, dlt, rsg, g10, ones_v, Cacc = _host_prep(
        np.asarray(data), np.asarray(input_distros),
        np.asarray(dense_layer_weights))

    if _CACHED is None:
        _CACHED = _build_nc()
    nc = _CACHED

    in_maps = []
    for c in range(NCORES):
        in_maps.append({
            "wt": wt, "dlt": dlt, "rsg": rsg, "ones": ones_v,
            "g10": np.ascontiguousarray(g10[:, :, c * BC:(c + 1) * BC]),
        })
    res = run_bass_kernel_spmd(
        nc, in_maps, core_ids=list(range(NCORES)),
        trace=bool(int(os.environ.get("KERNEL_TRACE", "0"))))
    LAST_RESULTS = res
    cs = np.concatenate([res.results[c]["colsum"].reshape(-1)
                         for c in range(NCORES)])           # (B,)
    lnY = np.log(cs.astype(np.float64)) + Cacc
    LAST_LNY = lnY
    y = np.exp(lnY).astype(np.float32).reshape(B, 1)
    return y



# revision 3
# speedup vs baseline: 1.1906x; 1.1906x over previous
"""HMM window log-likelihood on 8 NeuronCores (data-parallel over batch).

Math (per batch column b): y[b] = exp(logsumexp_i x_T[b,i]) with a log-space
forward recursion; evaluated here as a linear-space BACKWARD recursion
    beta_L = 1;  c_t = em_t . beta_t;  beta_{t-1} = Wn_t^T c_t
    y[b] = sum_i c_0[i,b]
where Wn_t = row-softmax(w[t-1]) (rowsum folded on host) and
em_t[i,b] = L[i, bin(b,t)] * g[t] is the emission table with per-step
rescale scalars g[t] (host-computed from batch column 0 in f64) folded in,
shipped to the device as one fp8 tensor.  Device per step: one matmul
(beta) + one elementwise multiply (c), the multiply split across the DVE
and GpSimd engines over four independent batch sub-chains so the serial
recursion pipelines.  Device returns colsum[b]; host: lnY = log(colsum)+C.
True lnY ~ -584.6 underflows f32 to 0.0, exactly matching the reference.
"""
import sys, os
for p in ("/opt/trn_rl_repo",):
    if p not in sys.path:
        sys.path.insert(0, p)
import numpy as np
import ml_dtypes

from concourse import bass, bacc, mybir
from concourse.tile import TileContext
from concourse.bass_utils import run_bass_kernel_spmd

W, L, B, NB = 128, 256, 4096, 10
NCORES = 8
BC = B // NCORES          # 512 batch cols per core
# two independent sub-chains pipelined on the DVE (the only non-Act
# engine that can read PSUM; GpSimd has no PSUM port on TRN2)
CHAINS = (
    ("A", 0, 256, "vector"),
    ("B", 256, 512, "vector"),
)
TEB = 16                  # emission-block steps per DMA tile

LAST_LNY = None           # debug: device-derived lnY per batch col
LAST_RESULTS = None       # debug: raw BassKernelResults

_CACHED = None            # (nc,) build cache


def _build_nc():
    nc = bacc.Bacc("TRN2", target_bir_lowering=False, debug=False,
                   num_devices=NCORES)
    bf16, f32, fp8 = mybir.dt.bfloat16, mybir.dt.float32, mybir.dt.float8e4

    wt = nc.dram_tensor("wt", [W, L - 1, W], bf16, kind="ExternalInput")
    em = nc.dram_tensor("em", [W, L, BC], fp8, kind="ExternalInput")
    ones = nc.dram_tensor("ones", [W, 1], bf16, kind="ExternalInput")
    colsum = nc.dram_tensor("colsum", [1, BC], f32, kind="ExternalOutput")

    with TileContext(nc) as tc:
        with tc.sbuf_pool(name="sb", bufs=2) as sb, \
                tc.psum_pool(name="ps", bufs=2) as ps:
            ones_sb = sb.tile([W, 1], bf16, bufs=1)
            nc.sync.dma_start(ones_sb, ones.ap())

            # all 255 transition matrices resident; chunked DMAs in backward
            # order so the scan can start as soon as the tail chunk lands
            wt_sb = sb.tile([W, L - 1, W], bf16, bufs=1)
            for cc in range((L - 1 + 7) // 8 - 1, -1, -1):
                t0 = cc * 8
                cnt = min(8, L - 1 - t0)
                nc.sync.dma_start(wt_sb[:, t0:t0 + cnt, :],
                                  wt.ap()[:, t0:t0 + cnt, :])

            beta_ps = {name: None for name, _, _, _ in CHAINS}
            cs_ps = ps.tile([1, BC], f32, tag="cs", bufs=1)
            for blk in range(L // TEB - 1, -1, -1):
                em_sb = sb.tile([W, TEB, BC], fp8, tag="em", bufs=3)
                nc.sync.dma_start(
                    em_sb, em.ap()[:, blk * TEB:(blk + 1) * TEB, :])
                for ti in range(TEB - 1, -1, -1):
                    t = blk * TEB + ti
                    for name, lo, hi, eng_name in CHAINS:
                        eng = getattr(nc, eng_name)
                        c_sb = sb.tile([W, hi - lo], bf16, tag=f"c{name}",
                                       bufs=2)
                        if t == L - 1:
                            eng.tensor_copy(c_sb, em_sb[:, ti, lo:hi])
                        else:
                            eng.tensor_mul(c_sb, em_sb[:, ti, lo:hi],
                                           beta_ps[name])
                        if t > 0:
                            b_ps = ps.tile([W, hi - lo], f32, tag=f"b{name}",
                                           bufs=1)
                            nc.tensor.matmul(b_ps, wt_sb[:, t - 1, :], c_sb,
                                             start=True, stop=True)
                            beta_ps[name] = b_ps
                        else:
                            nc.tensor.matmul(cs_ps[:, lo:hi], ones_sb, c_sb,
                                             start=True, stop=True)

            cs_sb = sb.tile([1, BC], f32, bufs=1)
            nc.vector.tensor_copy(cs_sb, cs_ps)
            nc.sync.dma_start(colsum.ap(), cs_sb)
    nc.compile()
    return nc


def _host_prep(data, input_distros, dense_layer_weights):
    f64 = np.float64
    w = dense_layer_weights.astype(f64)                    # (255,W,W)
    w = w - w.max(axis=2, keepdims=True)
    we = np.exp(w)
    wn = we / we.sum(axis=2, keepdims=True)                # row-softmax
    d = input_distros.astype(f64)
    d = d - d.max(axis=1, keepdims=True)
    e = np.exp(d)
    Ls = e / e.sum(axis=1, keepdims=True)                  # (W,NB) softmax rows
    # bins exactly as reference: floor(v / 0.1) in f32
    bins = np.minimum(NB - 1, np.floor(
        data / np.float32(0.1)).astype(np.int32))          # (B,L)

    # column-0 f64 backward pass -> per-step rescale g[t], offset C
    beta = np.ones(W, dtype=f64)
    Cacc = 0.0
    g = np.ones(L, dtype=f64)
    for t in range(L - 1, 0, -1):
        c = Ls[np.arange(W), bins[0, t]] * beta
        tmp = wn[t - 1].T @ c
        f = tmp.max()
        g[t] = 1.0 / f
        Cacc += np.log(f)
        beta = tmp * g[t]

    wt = np.ascontiguousarray(
        wn.transpose(1, 0, 2)).astype(ml_dtypes.bfloat16)  # (W,255,W)

    # emission table with per-step scale folded: em[i,t,b] = Ls[i,bin]*g[t]
    emf = Ls[:, bins.T]                                    # (W, L, B) f64
    emf *= g[None, :, None]
    np.clip(emf, 0.0, 224.0, out=emf)
    em8 = emf.astype(ml_dtypes.float8_e4m3)                # (W, L, B)

    ones_v = np.ones((W, 1), dtype=ml_dtypes.bfloat16)
    return wt, em8, ones_v, Cacc


def kernel(data, input_distros, dense_layer_weights):
    global LAST_LNY, LAST_RESULTS, _CACHED
    wt, em8, ones_v, Cacc = _host_prep(
        np.asarray(data), np.asarray(input_distros),
        np.asarray(dense_layer_weights))

    if _CACHED is None:
        _CACHED = _build_nc()
    nc = _CACHED

    in_maps = []
    for c in range(NCORES):
        in_maps.append({
            "wt": wt, "ones": ones_v,
            "em": np.ascontiguousarray(em8[:, :, c * BC:(c + 1) * BC]),
        })
    res = run_bass_kernel_spmd(
        nc, in_maps, core_ids=list(range(NCORES)),
        trace=bool(int(os.environ.get("KERNEL_TRACE", "0"))))
    LAST_RESULTS = res
    cs = np.concatenate([res.results[c]["colsum"].reshape(-1)
                         for c in range(NCORES)])           # (B,)
    with np.errstate(divide="ignore", invalid="ignore"):
        lnY = np.log(cs.astype(np.float64)) + Cacc
    LAST_LNY = lnY
    y = np.where(np.isfinite(lnY), np.exp(lnY), 0.0)
    y = y.astype(np.float32).reshape(B, 1)
    return y


# revision 6
# speedup vs baseline: 1.3915x; 1.1688x over previous
"""HMM window log-likelihood on 8 NeuronCores (data-parallel over batch).

Math (per batch column b): y[b] = exp(logsumexp_i x_T[b,i]) with a log-space
forward recursion; evaluated here as a linear-space BACKWARD recursion
    beta_L = 1;  c_t = em_t . beta_t;  beta_{t-1} = Wn_t^T c_t
    y[b] = sum_i c_0[i,b]
where Wn_t = row-softmax(w[t-1]) (rowsum folded on host into wt) and
em_t[i,b] = L[i, bin(b,t)] * g[t] is the emission table with per-step
rescale scalars g[t] (host-computed from batch column 0 in f64) folded in,
shipped to the device as one fp8 tensor (SBUF-resident stream, since the
DVE multiply may read at most one PSUM operand).  Device per step and per
256-column chain: c = em_sb * beta_ps (one DVE multiply, the only
elementwise op) -> beta matmul (PE).  Two independent chains pipeline the
serial recursion across PE/DVE; dummy matmuls keep the Tensor engine's
DVFS p-state high so the beta matmuls on the critical path stay short.
Device returns colsum[b]; host: lnY = log(colsum)+C.  True lnY ~ -584.6
underflows f32 to 0.0, exactly matching the reference.
"""
import sys, os
for p in ("/opt/trn_rl_repo",):
    if p not in sys.path:
        sys.path.insert(0, p)
import numpy as np
import ml_dtypes

from concourse import bass, bacc, mybir
from concourse.tile import TileContext
from concourse.bass_utils import run_bass_kernel_spmd

W, L, B, NB = 128, 256, 4096, 10
NCORES = 8
BC = B // NCORES          # 512 batch cols per core
# two independent sub-chains pipelined on the DVE (the only non-Act
# engine that can read PSUM; GpSimd has no PSUM port on TRN2)
CHAINS = (
    ("A", 0, 256),
    ("B", 256, 512),
)
TEB = 16                  # emission-block steps per DMA tile
DUMW = int(os.environ.get("KERNEL_DUMW", "512"))   # dummy matmul width

LAST_LNY = None           # debug: device-derived lnY per batch col
LAST_RESULTS = None       # debug: raw BassKernelResults

_CACHED = None            # (nc,) build cache


def _build_nc():
    nc = bacc.Bacc("TRN2", target_bir_lowering=False, debug=False,
                   num_devices=NCORES)
    bf16, f32, fp8 = mybir.dt.bfloat16, mybir.dt.float32, mybir.dt.float8e4

    wt = nc.dram_tensor("wt", [W, L - 1, W], bf16, kind="ExternalInput")
    em = nc.dram_tensor("em", [W, L, BC], fp8, kind="ExternalInput")
    ones = nc.dram_tensor("ones", [W, 1], bf16, kind="ExternalInput")
    colsum = nc.dram_tensor("colsum", [1, BC], f32, kind="ExternalOutput")

    with TileContext(nc) as tc:
        with tc.sbuf_pool(name="sb", bufs=2) as sb, \
                tc.psum_pool(name="ps", bufs=2) as ps:
            ones_sb = sb.tile([W, 1], bf16, bufs=1)
            nc.sync.dma_start(ones_sb, ones.ap())
            dum_sb = sb.tile([W, DUMW], bf16, bufs=1)
            nc.gpsimd.memset(dum_sb, 0.0)

            # all 255 transition matrices resident; chunked DMAs in backward
            # order so the scan can start as soon as the tail chunk lands
            wt_sb = sb.tile([W, L - 1, W], bf16, bufs=1)
            for cc in range((L - 1 + 7) // 8 - 1, -1, -1):
                t0 = cc * 8
                cnt = min(8, L - 1 - t0)
                nc.sync.dma_start(wt_sb[:, t0:t0 + cnt, :],
                                  wt.ap()[:, t0:t0 + cnt, :])

            dum_ps = ps.tile([W, DUMW], f32, tag="dum", bufs=1)

            def dummy_mm():
                # p-state filler: result never read; WAW on dum_ps only
                nc.tensor.matmul(dum_ps, wt_sb[:, L - 2, :],
                                 dum_sb, start=True, stop=True,
                                 skip_group_check=True)

            # pre-ramp the PE while input DMAs land
            for _ in range(8):
                dummy_mm()

            beta_ps = {}
            cs_ps = ps.tile([1, BC], f32, tag="cs", bufs=1)
            for blk in range(L // TEB - 1, -1, -1):
                em_sb = sb.tile([W, TEB, BC], fp8, tag="em", bufs=3)
                nc.sync.dma_start(
                    em_sb, em.ap()[:, blk * TEB:(blk + 1) * TEB, :])
                for ti in range(TEB - 1, -1, -1):
                    t = blk * TEB + ti
                    c_sb = {}
                    for name, lo, hi in CHAINS:
                        c = sb.tile([W, hi - lo], bf16, tag=f"c{name}",
                                    bufs=2)
                        if t == L - 1:
                            nc.vector.tensor_copy(c, em_sb[:, ti, lo:hi])
                        else:
                            nc.vector.tensor_mul(c, em_sb[:, ti, lo:hi],
                                                 beta_ps[name])
                        c_sb[name] = c
                    if DUMW:
                        dummy_mm()
                    for name, lo, hi in CHAINS:
                        if t > 0:
                            b_ps = ps.tile([W, hi - lo], f32, tag=f"b{name}",
                                           bufs=1)
                            nc.tensor.matmul(b_ps, wt_sb[:, t - 1, :],
                                             c_sb[name], start=True,
                                             stop=True)
                            beta_ps[name] = b_ps
                        else:
                            nc.tensor.matmul(cs_ps[:, lo:hi], ones_sb,
                                             c_sb[name], start=True,
                                             stop=True)

            cs_sb = sb.tile([1, BC], f32, bufs=1)
            nc.vector.tensor_copy(cs_sb, cs_ps)
            nc.sync.dma_start(colsum.ap(), cs_sb)
    nc.compile()
    return nc


def _host_prep(data, input_distros, dense_layer_weights):
    f64 = np.float64
    w = dense_layer_weights.astype(f64)                    # (255,W,W)
    w = w - w.max(axis=2, keepdims=True)
    we = np.exp(w)
    wn = we / we.sum(axis=2, keepdims=True)                # row-softmax
    d = input_distros.astype(f64)
    d = d - d.max(axis=1, keepdims=True)
    e = np.exp(d)
    Ls = e / e.sum(axis=1, keepdims=True)                  # (W,NB) softmax rows
    # bins exactly as reference: floor(v / 0.1) in f32
    bins = np.minimum(NB - 1, np.floor(
        data / np.float32(0.1)).astype(np.int32))          # (B,L)

    # column-0 f64 backward pass -> per-step rescale g[t], offset C
    beta = np.ones(W, dtype=f64)
    Cacc = 0.0
    g = np.ones(L, dtype=f64)
    for t in range(L - 1, 0, -1):
        c = Ls[np.arange(W), bins[0, t]] * beta
        tmp = wn[t - 1].T @ c
        f = tmp.max()
        g[t] = 1.0 / f
        Cacc += np.log(f)
        beta = tmp * g[t]

    wt = np.ascontiguousarray(
        wn.transpose(1, 0, 2)).astype(ml_dtypes.bfloat16)  # (W,255,W)

    # emission table with per-step scale folded: em[i,t,b] = Ls[i,bin]*g[t]
    emf = Ls[:, bins.T]                                    # (W, L, B) f64
    emf *= g[None, :, None]
    np.clip(emf, 0.0, 224.0, out=emf)
    em8 = emf.astype(ml_dtypes.float8_e4m3)                # (W, L, B)

    ones_v = np.ones((W, 1), dtype=ml_dtypes.bfloat16)
    return wt, em8, ones_v, Cacc


def kernel(data, input_distros, dense_layer_weights):
    global LAST_LNY, LAST_RESULTS, _CACHED
    wt, em8, ones_v, Cacc = _host_prep(
        np.asarray(data), np.asarray(input_distros),
        np.asarray(dense_layer_weights))

    if _CACHED is None:
        _CACHED = _build_nc()
    nc = _CACHED

    in_maps = []
    for c in range(NCORES):
        in_maps.append({
            "wt": wt, "ones": ones_v,
            "em": np.ascontiguousarray(em8[:, :, c * BC:(c + 1) * BC]),
        })
    res = run_bass_kernel_spmd(
        nc, in_maps, core_ids=list(range(NCORES)),
        trace=bool(int(os.environ.get("KERNEL_TRACE", "0"))))
    LAST_RESULTS = res
    cs = np.concatenate([res.results[c]["colsum"].reshape(-1)
                         for c in range(NCORES)])           # (B,)
    with np.errstate(divide="ignore", invalid="ignore"):
        lnY = np.log(cs.astype(np.float64)) + Cacc
    LAST_LNY = lnY
    y = np.where(np.isfinite(lnY), np.exp(lnY), 0.0)
    y = y.astype(np.float32).reshape(B, 1)
    return y


# revision 7
# speedup vs baseline: 1.3973x; 1.0042x over previous
"""HMM window log-likelihood on 8 NeuronCores (data-parallel over batch).

Math (per batch column b): y[b] = exp(logsumexp_i x_T[b,i]) with a log-space
forward recursion; evaluated here as a linear-space BACKWARD recursion
    beta_L = 1;  c_t = em_t . beta_t;  beta_{t-1} = Wn_t^T c_t
    y[b] = sum_i c_0[i,b]
where Wn_t = row-softmax(w[t-1]) (rowsum folded on host into wt) and
em_t[i,b] = L[i, bin(b,t)] * g[t] is the emission table with per-step
rescale scalars g[t] (host-computed from batch column 0 in f64) folded in,
shipped to the device as one fp8 tensor (SBUF-resident stream, since the
DVE multiply may read at most one PSUM operand).  Device per step and per
256-column chain: c = em_sb * beta_ps (one DVE multiply, the only
elementwise op) -> beta matmul (PE).  Two independent chains pipeline the
serial recursion across PE/DVE; dummy matmuls keep the Tensor engine's
DVFS p-state high so the beta matmuls on the critical path stay short.
Device returns colsum[b]; host: lnY = log(colsum)+C.  True lnY ~ -584.6
underflows f32 to 0.0, exactly matching the reference.
"""
import sys, os
for p in ("/opt/trn_rl_repo",):
    if p not in sys.path:
        sys.path.insert(0, p)
import numpy as np
import ml_dtypes

from concourse import bass, bacc, mybir
from concourse.tile import TileContext
from concourse.bass_utils import run_bass_kernel_spmd

W, L, B, NB = 128, 256, 4096, 10
NCORES = 8
BC = B // NCORES          # 512 batch cols per core
# two independent sub-chains pipelined on the DVE (the only non-Act
# engine that can read PSUM; GpSimd has no PSUM port on TRN2)
CHAINS = (
    ("A", 0, 256),
    ("B", 256, 512),
)
TEB = 16                  # emission-block steps per DMA tile
DUMW = int(os.environ.get("KERNEL_DUMW", "512"))   # dummy matmul width

LAST_LNY = None           # debug: device-derived lnY per batch col
LAST_RESULTS = None       # debug: raw BassKernelResults

_CACHED = None            # (nc,) build cache


def _build_nc():
    nc = bacc.Bacc("TRN2", target_bir_lowering=False, debug=False,
                   num_devices=NCORES)
    bf16, f32, fp8 = mybir.dt.bfloat16, mybir.dt.float32, mybir.dt.float8e4

    wt = nc.dram_tensor("wt", [W, L - 1, W], bf16, kind="ExternalInput")
    em = nc.dram_tensor("em", [W, L, BC], fp8, kind="ExternalInput")
    ones = nc.dram_tensor("ones", [W, 1], bf16, kind="ExternalInput")
    colsum = nc.dram_tensor("colsum", [1, BC], f32, kind="ExternalOutput")

    with TileContext(nc) as tc:
        with tc.sbuf_pool(name="sb", bufs=2) as sb, \
                tc.psum_pool(name="ps", bufs=2) as ps:
            ones_sb = sb.tile([W, 1], bf16, bufs=1)
            nc.sync.dma_start(ones_sb, ones.ap())
            dum_sb = sb.tile([W, DUMW], bf16, bufs=1)
            nc.gpsimd.memset(dum_sb, 0.0)

            # all 255 transition matrices resident; chunked DMAs in backward
            # order so the scan can start as soon as the tail chunk lands
            wt_sb = sb.tile([W, L - 1, W], bf16, bufs=1)
            for cc in range((L - 1 + 7) // 8 - 1, -1, -1):
                t0 = cc * 8
                cnt = min(8, L - 1 - t0)
                nc.sync.dma_start(wt_sb[:, t0:t0 + cnt, :],
                                  wt.ap()[:, t0:t0 + cnt, :])

            dum_ps = ps.tile([W, DUMW], f32, tag="dum", bufs=1)

            def dummy_mm():
                # p-state filler: result never read; WAW on dum_ps only
                nc.tensor.matmul(dum_ps, wt_sb[:, L - 2, :],
                                 dum_sb, start=True, stop=True,
                                 skip_group_check=True)

            # pre-ramp the PE while input DMAs land
            for _ in range(8):
                dummy_mm()

            beta_ps = {}
            cs_ps = ps.tile([1, BC], f32, tag="cs", bufs=1)
            for blk in range(L // TEB - 1, -1, -1):
                em_sb = sb.tile([W, TEB, BC], fp8, tag="em", bufs=3)
                nc.sync.dma_start(
                    em_sb, em.ap()[:, blk * TEB:(blk + 1) * TEB, :])
                for ti in range(TEB - 1, -1, -1):
                    t = blk * TEB + ti
                    c_sb = {}
                    for name, lo, hi in CHAINS:
                        c = sb.tile([W, hi - lo], bf16, tag=f"c{name}",
                                    bufs=2)
                        if t == L - 1:
                            nc.vector.tensor_copy(c, em_sb[:, ti, lo:hi])
                        else:
                            nc.vector.tensor_mul(c, em_sb[:, ti, lo:hi],
                                                 beta_ps[name])
                        c_sb[name] = c
                    if DUMW:
                        dummy_mm()
                        dummy_mm()
                    for name, lo, hi in CHAINS:
                        if t > 0:
                            b_ps = ps.tile([W, hi - lo], f32, tag=f"b{name}",
                                           bufs=1)
                            nc.tensor.matmul(b_ps, wt_sb[:, t - 1, :],
                                             c_sb[name], start=True,
                                             stop=True)
                            beta_ps[name] = b_ps
                        else:
                            nc.tensor.matmul(cs_ps[:, lo:hi], ones_sb,
                                             c_sb[name], start=True,
                                             stop=True)

            cs_sb = sb.tile([1, BC], f32, bufs=1)
            nc.vector.tensor_copy(cs_sb, cs_ps)
            nc.sync.dma_start(colsum.ap(), cs_sb)
    nc.compile()
    return nc


def _host_prep(data, input_distros, dense_layer_weights):
    f64 = np.float64
    w = dense_layer_weights.astype(f64)                    # (255,W,W)
    w = w - w.max(axis=2, keepdims=True)
    we = np.exp(w)
    wn = we / we.sum(axis=2, keepdims=True)                # row-softmax
    d = input_distros.astype(f64)
    d = d - d.max(axis=1, keepdims=True)
    e = np.exp(d)
    Ls = e / e.sum(axis=1, keepdims=True)                  # (W,NB) softmax rows
    # bins exactly as reference: floor(v / 0.1) in f32
    bins = np.minimum(NB - 1, np.floor(
        data / np.float32(0.1)).astype(np.int32))          # (B,L)

    # column-0 f64 backward pass -> per-step rescale g[t], offset C
    beta = np.ones(W, dtype=f64)
    Cacc = 0.0
    g = np.ones(L, dtype=f64)
    for t in range(L - 1, 0, -1):
        c = Ls[np.arange(W), bins[0, t]] * beta
        tmp = wn[t - 1].T @ c
        f = tmp.max()
        g[t] = 1.0 / f
        Cacc += np.log(f)
        beta = tmp * g[t]

    wt = np.ascontiguousarray(
        wn.transpose(1, 0, 2)).astype(ml_dtypes.bfloat16)  # (W,255,W)

    # emission table with per-step scale folded: em[i,t,b] = Ls[i,bin]*g[t]
    emf = Ls[:, bins.T]                                    # (W, L, B) f64
    emf *= g[None, :, None]
    np.clip(emf, 0.0, 224.0, out=emf)
    em8 = emf.astype(ml_dtypes.float8_e4m3)                # (W, L, B)

    ones_v = np.ones((W, 1), dtype=ml_dtypes.bfloat16)
    return wt, em8, ones_v, Cacc


def kernel(data, input_distros, dense_layer_weights):
    global LAST_LNY, LAST_RESULTS, _CACHED
    wt, em8, ones_v, Cacc = _host_prep(
        np.asarray(data), np.asarray(input_distros),
        np.asarray(dense_layer_weights))

    if _CACHED is None:
        _CACHED = _build_nc()
    nc = _CACHED

    in_maps = []
    for c in range(NCORES):
        in_maps.append({
            "wt": wt, "ones": ones_v,
            "em": np.ascontiguousarray(em8[:, :, c * BC:(c + 1) * BC]),
        })
    res = run_bass_kernel_spmd(
        nc, in_maps, core_ids=list(range(NCORES)),
        trace=bool(int(os.environ.get("KERNEL_TRACE", "0"))))
    LAST_RESULTS = res
    cs = np.concatenate([res.results[c]["colsum"].reshape(-1)
                         for c in range(NCORES)])           # (B,)
    with np.errstate(divide="ignore", invalid="ignore"):
        lnY = np.log(cs.astype(np.float64)) + Cacc
    LAST_LNY = lnY
    y = np.where(np.isfinite(lnY), np.exp(lnY), 0.0)
    y = y.astype(np.float32).reshape(B, 1)
    return y


# revision 8
# speedup vs baseline: 1.4033x; 1.0043x over previous
"""HMM window log-likelihood on 8 NeuronCores (data-parallel over batch).

Math (per batch column b): y[b] = exp(logsumexp_i x_T[b,i]) with a log-space
forward recursion; evaluated here as a linear-space BACKWARD recursion
    beta_L = 1;  c_t = em_t . beta_t;  beta_{t-1} = Wn_t^T c_t
    y[b] = sum_i c_0[i,b]
where Wn_t = row-softmax(w[t-1]) (rowsum folded on host into wt) and
em_t[i,b] = L[i, bin(b,t)] * g[t] is the emission table with per-step
rescale scalars g[t] (host-computed from batch column 0 in f64) folded in,
shipped to the device as one fp8 tensor (SBUF-resident stream, since the
DVE multiply may read at most one PSUM operand).  Device per step and per
256-column chain: c = em_sb * beta_ps (one DVE multiply, the only
elementwise op) -> beta matmul (PE).  Two independent chains pipeline the
serial recursion across PE/DVE; dummy matmuls keep the Tensor engine's
DVFS p-state high so the beta matmuls on the critical path stay short.
Device returns colsum[b]; host: lnY = log(colsum)+C.  True lnY ~ -584.6
underflows f32 to 0.0, exactly matching the reference.
"""
import sys, os
for p in ("/opt/trn_rl_repo",):
    if p not in sys.path:
        sys.path.insert(0, p)
import numpy as np
import ml_dtypes

from concourse import bass, bacc, mybir
from concourse.tile import TileContext
from concourse.bass_utils import run_bass_kernel_spmd

W, L, B, NB = 128, 256, 4096, 10
NCORES = 8
BC = B // NCORES          # 512 batch cols per core
# two independent sub-chains pipelined on the DVE (the only non-Act
# engine that can read PSUM; GpSimd has no PSUM port on TRN2)
CHAINS = (
    ("A", 0, 256),
    ("B", 256, 512),
)
TEB = 16                  # emission-block steps per DMA tile
DUMW = int(os.environ.get("KERNEL_DUMW", "512"))   # dummy matmul width

LAST_LNY = None           # debug: device-derived lnY per batch col
LAST_RESULTS = None       # debug: raw BassKernelResults

_CACHED = None            # (nc,) build cache


def _build_nc():
    nc = bacc.Bacc("TRN2", target_bir_lowering=False, debug=False,
                   num_devices=NCORES)
    bf16, f32, fp8 = mybir.dt.bfloat16, mybir.dt.float32, mybir.dt.float8e4

    wt = nc.dram_tensor("wt", [W, L - 1, W], bf16, kind="ExternalInput")
    em = nc.dram_tensor("em", [W, L, BC], fp8, kind="ExternalInput")
    ones = nc.dram_tensor("ones", [W, 1], bf16, kind="ExternalInput")
    colsum = nc.dram_tensor("colsum", [1, BC], f32, kind="ExternalOutput")

    with TileContext(nc) as tc:
        with tc.sbuf_pool(name="sb", bufs=2) as sb, \
                tc.psum_pool(name="ps", bufs=2) as ps:
            ones_sb = sb.tile([W, 1], bf16, bufs=1)
            nc.sync.dma_start(ones_sb, ones.ap())
            dum_sb = sb.tile([W, DUMW], bf16, bufs=1)
            nc.gpsimd.memset(dum_sb, 0.0)

            # all 255 transition matrices resident; chunked DMAs in backward
            # order so the scan can start as soon as the tail chunk lands
            wt_sb = sb.tile([W, L - 1, W], bf16, bufs=1)
            for cc in range((L - 1 + 7) // 8 - 1, -1, -1):
                t0 = cc * 8
                cnt = min(8, L - 1 - t0)
                nc.sync.dma_start(wt_sb[:, t0:t0 + cnt, :],
                                  wt.ap()[:, t0:t0 + cnt, :])

            dum_ps = ps.tile([W, DUMW], f32, tag="dum", bufs=1)

            def dummy_mm():
                # p-state filler: result never read; WAW on dum_ps only
                nc.tensor.matmul(dum_ps, wt_sb[:, L - 2, :],
                                 dum_sb, start=True, stop=True,
                                 skip_group_check=True)

            # pre-ramp the PE while input DMAs land
            for _ in range(8):
                dummy_mm()

            beta_ps = {}
            cs_ps = ps.tile([1, BC], f32, tag="cs", bufs=1)
            for blk in range(L // TEB - 1, -1, -1):
                em_sb = sb.tile([W, TEB, BC], fp8, tag="em", bufs=3)
                nc.sync.dma_start(
                    em_sb, em.ap()[:, blk * TEB:(blk + 1) * TEB, :])
                for ti in range(TEB - 1, -1, -1):
                    t = blk * TEB + ti
                    c_sb = {}
                    for name, lo, hi in CHAINS:
                        c = sb.tile([W, hi - lo], bf16, tag=f"c{name}",
                                    bufs=2)
                        if t == L - 1:
                            nc.vector.tensor_copy(c, em_sb[:, ti, lo:hi])
                        else:
                            nc.vector.tensor_mul(c, em_sb[:, ti, lo:hi],
                                                 beta_ps[name])
                        c_sb[name] = c
                    for name, lo, hi in CHAINS:
                        if t > 0:
                            b_ps = ps.tile([W, hi - lo], f32, tag=f"b{name}",
                                           bufs=1)
                            nc.tensor.matmul(b_ps, wt_sb[:, t - 1, :],
                                             c_sb[name], start=True,
                                             stop=True)
                            beta_ps[name] = b_ps
                        else:
                            nc.tensor.matmul(cs_ps[:, lo:hi], ones_sb,
                                             c_sb[name], start=True,
                                             stop=True)
                    if DUMW:
                        dummy_mm()
                        dummy_mm()

            cs_sb = sb.tile([1, BC], f32, bufs=1)
            nc.vector.tensor_copy(cs_sb, cs_ps)
            nc.sync.dma_start(colsum.ap(), cs_sb)
    nc.compile()
    return nc


def _host_prep(data, input_distros, dense_layer_weights):
    f64 = np.float64
    w = dense_layer_weights.astype(f64)                    # (255,W,W)
    w = w - w.max(axis=2, keepdims=True)
    we = np.exp(w)
    wn = we / we.sum(axis=2, keepdims=True)                # row-softmax
    d = input_distros.astype(f64)
    d = d - d.max(axis=1, keepdims=True)
    e = np.exp(d)
    Ls = e / e.sum(axis=1, keepdims=True)                  # (W,NB) softmax rows
    # bins exactly as reference: floor(v / 0.1) in f32
    bins = np.minimum(NB - 1, np.floor(
        data / np.float32(0.1)).astype(np.int32))          # (B,L)

    # column-0 f64 backward pass -> per-step rescale g[t], offset C
    beta = np.ones(W, dtype=f64)
    Cacc = 0.0
    g = np.ones(L, dtype=f64)
    for t in range(L - 1, 0, -1):
        c = Ls[np.arange(W), bins[0, t]] * beta
        tmp = wn[t - 1].T @ c
        f = tmp.max()
        g[t] = 1.0 / f
        Cacc += np.log(f)
        beta = tmp * g[t]

    wt = np.ascontiguousarray(
        wn.transpose(1, 0, 2)).astype(ml_dtypes.bfloat16)  # (W,255,W)

    # emission table with per-step scale folded: em[i,t,b] = Ls[i,bin]*g[t]
    emf = Ls[:, bins.T]                                    # (W, L, B) f64
    emf *= g[None, :, None]
    np.clip(emf, 0.0, 224.0, out=emf)
    em8 = emf.astype(ml_dtypes.float8_e4m3)                # (W, L, B)

    ones_v = np.ones((W, 1), dtype=ml_dtypes.bfloat16)
    return wt, em8, ones_v, Cacc


def kernel(data, input_distros, dense_layer_weights):
    global LAST_LNY, LAST_RESULTS, _CACHED
    wt, em8, ones_v, Cacc = _host_prep(
        np.asarray(data), np.asarray(input_distros),
        np.asarray(dense_layer_weights))

    if _CACHED is None:
        _CACHED = _build_nc()
    nc = _CACHED

    in_maps = []
    for c in range(NCORES):
        in_maps.append({
            "wt": wt, "ones": ones_v,
            "em": np.ascontiguousarray(em8[:, :, c * BC:(c + 1) * BC]),
        })
    res = run_bass_kernel_spmd(
        nc, in_maps, core_ids=list(range(NCORES)),
        trace=bool(int(os.environ.get("KERNEL_TRACE", "0"))))
    LAST_RESULTS = res
    cs = np.concatenate([res.results[c]["colsum"].reshape(-1)
                         for c in range(NCORES)])           # (B,)
    with np.errstate(divide="ignore", invalid="ignore"):
        lnY = np.log(cs.astype(np.float64)) + Cacc
    LAST_LNY = lnY
    y = np.where(np.isfinite(lnY), np.exp(lnY), 0.0)
    y = y.astype(np.float32).reshape(B, 1)
    return y


# revision 9
# speedup vs baseline: 1.4189x; 1.0111x over previous
"""HMM window log-likelihood on 8 NeuronCores (data-parallel over batch).

Math (per batch column b): y[b] = exp(logsumexp_i x_T[b,i]) with a log-space
forward recursion; evaluated here as a linear-space BACKWARD recursion
    beta_L = 1;  c_t = em_t . beta_t;  beta_{t-1} = Wn_t^T c_t
    y[b] = sum_i c_0[i,b]
where Wn_t = row-softmax(w[t-1]) (rowsum folded on host into wt) and
em_t[i,b] = L[i, bin(b,t)] * g[t] is the emission table with per-step
rescale scalars g[t] (host-computed from batch column 0 in f64) folded in,
shipped to the device as one fp8 tensor (SBUF-resident stream, since the
DVE multiply may read at most one PSUM operand).  Device per step and per
256-column chain: c = em_sb * beta_ps (one DVE multiply, the only
elementwise op) -> beta matmul (PE).  Two independent chains pipeline the
serial recursion across PE/DVE; dummy matmuls keep the Tensor engine's
DVFS p-state high so the beta matmuls on the critical path stay short.
Device returns colsum[b]; host: lnY = log(colsum)+C.  True lnY ~ -584.6
underflows f32 to 0.0, exactly matching the reference.
"""
import sys, os
for p in ("/opt/trn_rl_repo",):
    if p not in sys.path:
        sys.path.insert(0, p)
import numpy as np
import ml_dtypes

from concourse import bass, bacc, mybir
from concourse.tile import TileContext
from concourse.bass_utils import run_bass_kernel_spmd

W, L, B, NB = 128, 256, 4096, 10
NCORES = 8
BC = B // NCORES          # 512 batch cols per core
# two independent sub-chains pipelined on the DVE (the only non-Act
# engine that can read PSUM; GpSimd has no PSUM port on TRN2)
CHAINS = (
    ("A", 0, 256),
    ("B", 256, 512),
)
TEB = 16                  # emission-block steps per DMA tile
DUMW = int(os.environ.get("KERNEL_DUMW", "160"))   # dummy matmul width
DUMN = int(os.environ.get("KERNEL_DUMN", "5"))     # dummy matmuls per step

LAST_LNY = None           # debug: device-derived lnY per batch col
LAST_RESULTS = None       # debug: raw BassKernelResults

_CACHED = None            # (nc,) build cache


def _build_nc():
    nc = bacc.Bacc("TRN2", target_bir_lowering=False, debug=False,
                   num_devices=NCORES)
    bf16, f32, fp8 = mybir.dt.bfloat16, mybir.dt.float32, mybir.dt.float8e4

    wt = nc.dram_tensor("wt", [W, L - 1, W], bf16, kind="ExternalInput")
    em = nc.dram_tensor("em", [W, L, BC], fp8, kind="ExternalInput")
    ones = nc.dram_tensor("ones", [W, 1], bf16, kind="ExternalInput")
    colsum = nc.dram_tensor("colsum", [1, BC], f32, kind="ExternalOutput")

    with TileContext(nc) as tc:
        with tc.sbuf_pool(name="sb", bufs=2) as sb, \
                tc.psum_pool(name="ps", bufs=2) as ps:
            ones_sb = sb.tile([W, 1], bf16, bufs=1)
            nc.sync.dma_start(ones_sb, ones.ap())
            dum_sb = sb.tile([W, DUMW], bf16, bufs=1)
            nc.gpsimd.memset(dum_sb, 0.0)

            # all 255 transition matrices resident; chunked DMAs in backward
            # order so the scan can start as soon as the tail chunk lands
            wt_sb = sb.tile([W, L - 1, W], bf16, bufs=1)
            for cc in range((L - 1 + 7) // 8 - 1, -1, -1):
                t0 = cc * 8
                cnt = min(8, L - 1 - t0)
                nc.sync.dma_start(wt_sb[:, t0:t0 + cnt, :],
                                  wt.ap()[:, t0:t0 + cnt, :])

            dum_ps = ps.tile([W, DUMW], f32, tag="dum", bufs=1)

            def dummy_mm():
                # p-state filler: result never read; WAW on dum_ps only
                nc.tensor.matmul(dum_ps, wt_sb[:, L - 2, :],
                                 dum_sb, start=True, stop=True,
                                 skip_group_check=True)

            # pre-ramp the PE while input DMAs land
            for _ in range(8):
                dummy_mm()

            beta_ps = {}
            cs_ps = ps.tile([1, BC], f32, tag="cs", bufs=1)
            for blk in range(L // TEB - 1, -1, -1):
                em_sb = sb.tile([W, TEB, BC], fp8, tag="em", bufs=3)
                nc.sync.dma_start(
                    em_sb, em.ap()[:, blk * TEB:(blk + 1) * TEB, :])
                for ti in range(TEB - 1, -1, -1):
                    t = blk * TEB + ti
                    c_sb = {}
                    for name, lo, hi in CHAINS:
                        c = sb.tile([W, hi - lo], bf16, tag=f"c{name}",
                                    bufs=2)
                        if t == L - 1:
                            nc.vector.tensor_copy(c, em_sb[:, ti, lo:hi])
                        else:
                            nc.vector.tensor_mul(c, em_sb[:, ti, lo:hi],
                                                 beta_ps[name])
                        c_sb[name] = c
                    for name, lo, hi in CHAINS:
                        if t > 0:
                            b_ps = ps.tile([W, hi - lo], f32, tag=f"b{name}",
                                           bufs=1)
                            nc.tensor.matmul(b_ps, wt_sb[:, t - 1, :],
                                             c_sb[name], start=True,
                                             stop=True)
                            beta_ps[name] = b_ps
                        else:
                            nc.tensor.matmul(cs_ps[:, lo:hi], ones_sb,
                                             c_sb[name], start=True,
                                             stop=True)
                    for _ in range(DUMN if DUMW else 0):
                        dummy_mm()

            cs_sb = sb.tile([1, BC], f32, bufs=1)
            nc.vector.tensor_copy(cs_sb, cs_ps)
            nc.sync.dma_start(colsum.ap(), cs_sb)
    nc.compile()
    return nc


def _host_prep(data, input_distros, dense_layer_weights):
    f64 = np.float64
    w = dense_layer_weights.astype(f64)                    # (255,W,W)
    w = w - w.max(axis=2, keepdims=True)
    we = np.exp(w)
    wn = we / we.sum(axis=2, keepdims=True)                # row-softmax
    d = input_distros.astype(f64)
    d = d - d.max(axis=1, keepdims=True)
    e = np.exp(d)
    Ls = e / e.sum(axis=1, keepdims=True)                  # (W,NB) softmax rows
    # bins exactly as reference: floor(v / 0.1) in f32
    bins = np.minimum(NB - 1, np.floor(
        data / np.float32(0.1)).astype(np.int32))          # (B,L)

    # column-0 f64 backward pass -> per-step rescale g[t], offset C
    beta = np.ones(W, dtype=f64)
    Cacc = 0.0
    g = np.ones(L, dtype=f64)
    for t in range(L - 1, 0, -1):
        c = Ls[np.arange(W), bins[0, t]] * beta
        tmp = wn[t - 1].T @ c
        f = tmp.max()
        g[t] = 1.0 / f
        Cacc += np.log(f)
        beta = tmp * g[t]

    wt = np.ascontiguousarray(
        wn.transpose(1, 0, 2)).astype(ml_dtypes.bfloat16)  # (W,255,W)

    # emission table with per-step scale folded: em[i,t,b] = Ls[i,bin]*g[t]
    emf = Ls[:, bins.T]                                    # (W, L, B) f64
    emf *= g[None, :, None]
    np.clip(emf, 0.0, 224.0, out=emf)
    em8 = emf.astype(ml_dtypes.float8_e4m3)                # (W, L, B)

    ones_v = np.ones((W, 1), dtype=ml_dtypes.bfloat16)
    return wt, em8, ones_v, Cacc


def kernel(data, input_distros, dense_layer_weights):
    global LAST_LNY, LAST_RESULTS, _CACHED
    wt, em8, ones_v, Cacc = _host_prep(
        np.asarray(data), np.asarray(input_distros),
        np.asarray(dense_layer_weights))

    if _CACHED is None:
        _CACHED = _build_nc()
    nc = _CACHED

    in_maps = []
    for c in range(NCORES):
        in_maps.append({
            "wt": wt, "ones": ones_v,
            "em": np.ascontiguousarray(em8[:, :, c * BC:(c + 1) * BC]),
        })
    res = run_bass_kernel_spmd(
        nc, in_maps, core_ids=list(range(NCORES)),
        trace=bool(int(os.environ.get("KERNEL_TRACE", "0"))))
    LAST_RESULTS = res
    cs = np.concatenate([res.results[c]["colsum"].reshape(-1)
                         for c in range(NCORES)])           # (B,)
    with np.errstate(divide="ignore", invalid="ignore"):
        lnY = np.log(cs.astype(np.float64)) + Cacc
    LAST_LNY = lnY
    y = np.where(np.isfinite(lnY), np.exp(lnY), 0.0)
    y = y.astype(np.float32).reshape(B, 1)
    return y


# revision 11
# speedup vs baseline: 1.5646x; 1.1027x over previous
"""HMM window log-likelihood on 8 NeuronCores (data-parallel over batch).

Math (per batch column b): y[b] = exp(logsumexp_i x_T[b,i]) with a log-space
forward recursion; evaluated here as a linear-space BACKWARD recursion
    beta_L = 1;  c_t = em_t . beta_t;  beta_{t-1} = Wn_t^T c_t
    y[b] = sum_i c_0[i,b]
where Wn_t = row-softmax(w[t-1]) (rowsum folded on host into wt) and
em_t[i,b] = L[i, bin(b,t)] * g[t] is the emission table with per-step
rescale scalars g[t] (host-computed from batch column 0 in f64) folded in,
shipped to the device as one fp8 tensor (SBUF-resident stream, since the
DVE multiply may read at most one PSUM operand).  Device per step and per
256-column chain: c = em_sb * beta_ps (one DVE multiply, the only
elementwise op) -> beta matmul (PE).  Two independent chains pipeline the
serial recursion across PE/DVE; dummy matmuls keep the Tensor engine's
DVFS p-state high so the beta matmuls on the critical path stay short.
Device returns colsum[b]; host: lnY = log(colsum)+C.  True lnY ~ -584.6
underflows f32 to 0.0, exactly matching the reference.
"""
import sys, os
for p in ("/opt/trn_rl_repo",):
    if p not in sys.path:
        sys.path.insert(0, p)
import numpy as np
import ml_dtypes

from concourse import bass, bacc, mybir
from concourse.tile import TileContext
from concourse.bass_utils import run_bass_kernel_spmd

W, L, B, NB = 128, 256, 4096, 10
NCORES = 8
BC = B // NCORES          # 512 batch cols per core
# two independent sub-chains pipelined on the DVE (the only non-Act
# engine that can read PSUM; GpSimd has no PSUM port on TRN2)
CHAINS = (
    ("A", 0, 256),
    ("B", 256, 512),
)
TEB = 16                  # emission-block steps per DMA tile
DUMW = int(os.environ.get("KERNEL_DUMW", "160"))   # dummy matmul width
DUMN = int(os.environ.get("KERNEL_DUMN", "5"))     # dummy matmuls per step

LAST_LNY = None           # debug: device-derived lnY per batch col
LAST_RESULTS = None       # debug: raw BassKernelResults

_CACHED = None            # (nc,) build cache


def _build_nc():
    nc = bacc.Bacc("TRN2", target_bir_lowering=False, debug=False,
                   num_devices=NCORES)
    bf16, f32, fp8 = mybir.dt.bfloat16, mybir.dt.float32, mybir.dt.float8e4

    wt = nc.dram_tensor("wt", [W, L - 1, W], bf16, kind="ExternalInput")
    em = nc.dram_tensor("em", [W, L, BC], fp8, kind="ExternalInput")
    ones = nc.dram_tensor("ones", [W, 1], bf16, kind="ExternalInput")
    colsum = nc.dram_tensor("colsum", [1, BC], f32, kind="ExternalOutput")

    with TileContext(nc) as tc:
        with tc.sbuf_pool(name="sb", bufs=2) as sb, \
                tc.psum_pool(name="ps", bufs=2) as ps:
            ones_sb = sb.tile([W, 1], bf16, bufs=1)
            nc.sync.dma_start(ones_sb, ones.ap())
            dum_sb = sb.tile([W, DUMW], bf16, bufs=1)
            nc.gpsimd.memset(dum_sb, 0.0)

            # all 255 transition matrices resident; chunked DMAs in backward
            # order so the scan can start as soon as the tail chunk lands.
            # The em-block DMA for the first (highest-t) steps is issued
            # between the first wt chunks so the recursion isn't stuck
            # behind the full 8 MB wt transfer.
            wt_sb = sb.tile([W, L - 1, W], bf16, bufs=1)
            em_tiles = {}

            def dma_em_block(blk):
                em_sb = sb.tile([W, TEB, BC], fp8, tag="em", bufs=3)
                nc.sync.dma_start(
                    em_sb, em.ap()[:, blk * TEB:(blk + 1) * TEB, :])
                em_tiles[blk] = em_sb

            NBLK = L // TEB
            dma_em_block(NBLK - 1)
            for cc in range((L - 1 + 7) // 8 - 1, -1, -1):
                t0 = cc * 8
                cnt = min(8, L - 1 - t0)
                nc.sync.dma_start(wt_sb[:, t0:t0 + cnt, :],
                                  wt.ap()[:, t0:t0 + cnt, :])
                if cc == (L - 1 + 7) // 8 - 3:
                    dma_em_block(NBLK - 2)

            dum_ps = ps.tile([W, DUMW], f32, tag="dum", bufs=1)

            def dummy_mm():
                # p-state filler: result never read; WAW on dum_ps only
                nc.tensor.matmul(dum_ps, wt_sb[:, L - 2, :],
                                 dum_sb, start=True, stop=True,
                                 skip_group_check=True)

            # pre-ramp the PE while input DMAs land
            for _ in range(8):
                dummy_mm()

            beta_ps = {}
            cs_ps = ps.tile([1, BC], f32, tag="cs", bufs=1)
            for blk in range(NBLK - 1, -1, -1):
                if blk not in em_tiles:
                    dma_em_block(blk)
                em_sb = em_tiles.pop(blk)
                for ti in range(TEB - 1, -1, -1):
                    t = blk * TEB + ti
                    c_sb = {}
                    for name, lo, hi in CHAINS:
                        c = sb.tile([W, hi - lo], bf16, tag=f"c{name}",
                                    bufs=2)
                        if t == L - 1:
                            nc.vector.tensor_copy(c, em_sb[:, ti, lo:hi])
                        else:
                            nc.vector.tensor_mul(c, em_sb[:, ti, lo:hi],
                                                 beta_ps[name])
                        c_sb[name] = c
                    for name, lo, hi in CHAINS:
                        if t > 0:
                            b_ps = ps.tile([W, hi - lo], f32, tag=f"b{name}",
                                           bufs=1)
                            nc.tensor.matmul(b_ps, wt_sb[:, t - 1, :],
                                             c_sb[name], start=True,
                                             stop=True)
                            beta_ps[name] = b_ps
                        else:
                            nc.tensor.matmul(cs_ps[:, lo:hi], ones_sb,
                                             c_sb[name], start=True,
                                             stop=True)
                    for _ in range(DUMN if DUMW else 0):
                        dummy_mm()

            cs_sb = sb.tile([1, BC], f32, bufs=1)
            nc.vector.tensor_copy(cs_sb, cs_ps)
            nc.sync.dma_start(colsum.ap(), cs_sb)
    nc.compile()
    return nc


def _host_prep(data, input_distros, dense_layer_weights):
    f64 = np.float64
    w = dense_layer_weights.astype(f64)                    # (255,W,W)
    w = w - w.max(axis=2, keepdims=True)
    we = np.exp(w)
    wn = we / we.sum(axis=2, keepdims=True)                # row-softmax
    d = input_distros.astype(f64)
    d = d - d.max(axis=1, keepdims=True)
    e = np.exp(d)
    Ls = e / e.sum(axis=1, keepdims=True)                  # (W,NB) softmax rows
    # bins exactly as reference: floor(v / 0.1) in f32
    bins = np.minimum(NB - 1, np.floor(
        data / np.float32(0.1)).astype(np.int32))          # (B,L)

    # column-0 f64 backward pass -> per-step rescale g[t], offset C
    beta = np.ones(W, dtype=f64)
    Cacc = 0.0
    g = np.ones(L, dtype=f64)
    for t in range(L - 1, 0, -1):
        c = Ls[np.arange(W), bins[0, t]] * beta
        tmp = wn[t - 1].T @ c
        f = tmp.max()
        g[t] = 1.0 / f
        Cacc += np.log(f)
        beta = tmp * g[t]

    wt = np.ascontiguousarray(
        wn.transpose(1, 0, 2)).astype(ml_dtypes.bfloat16)  # (W,255,W)

    # emission table with per-step scale folded: em[i,t,b] = Ls[i,bin]*g[t]
    emf = Ls[:, bins.T]                                    # (W, L, B) f64
    emf *= g[None, :, None]
    np.clip(emf, 0.0, 224.0, out=emf)
    em8 = emf.astype(ml_dtypes.float8_e4m3)                # (W, L, B)

    ones_v = np.ones((W, 1), dtype=ml_dtypes.bfloat16)
    return wt, em8, ones_v, Cacc


def kernel(data, input_distros, dense_layer_weights):
    global LAST_LNY, LAST_RESULTS, _CACHED
    wt, em8, ones_v, Cacc = _host_prep(
        np.asarray(data), np.asarray(input_distros),
        np.asarray(dense_layer_weights))

    if _CACHED is None:
        _CACHED = _build_nc()
    nc = _CACHED

    in_maps = []
    for c in range(NCORES):
        in_maps.append({
            "wt": wt, "ones": ones_v,
            "em": np.ascontiguousarray(em8[:, :, c * BC:(c + 1) * BC]),
        })
    res = run_bass_kernel_spmd(
        nc, in_maps, core_ids=list(range(NCORES)),
        trace=bool(int(os.environ.get("KERNEL_TRACE", "0"))))
    LAST_RESULTS = res
    cs = np.concatenate([res.results[c]["colsum"].reshape(-1)
                         for c in range(NCORES)])           # (B,)
    with np.errstate(divide="ignore", invalid="ignore"):
        lnY = np.log(cs.astype(np.float64)) + Cacc
    LAST_LNY = lnY
    y = np.where(np.isfinite(lnY), np.exp(lnY), 0.0)
    y = y.astype(np.float32).reshape(B, 1)
    return y
